# revision 4
# baseline (speedup 1.0000x reference)
"""IsoGMM loss kernel for 8 Trainium2 NeuronCores.

loss = mean_{n,k} r[n,k] * ||X[n] - mus[k]||^2

Decomposition (two accumulated PE matmuls per core over 64 DoubleRow
row-pair segments; fp8 e4m3 inputs, ~4e-3 rel err vs the 2e-2 budget):
  ps  [64,129] += r_pair.T @ [X|1]_pair        (M = r.T X, col 128 = C_k)
  ps2 [64,129] += r_pair.T @ ([X|1]^2)_pair    (row-sum -> T1 + C_k)
Host combine contracts ps with mus (O(K*D)) and reads the on-chip
row-reduction of ps2.

Pipeline structure (from perfetto/ntff trace analysis):
 - Host interleaves X and r into per-block contiguous pieces
   ([X(n x 129) | r(n x 64)] per partition), so one DMA delivers
   everything a run of pairs needs - no X/r queue competition, bigger
   per-partition lines.
 - Blocks are consumed in expected ARRIVAL order, not segment order
   (the loss is a sum; PSUM accumulation order is irrelevant). The sync
   queue prefetches two late blocks early; all gpsimd square work rides
   them (gpsimd needs ~2.8us lead per 12-seg slice).
 - DMA sems fire on the slowest of 16 engine rows, so under cross-queue
   contention the early window uses small 8-seg pieces round-robined
   across the scalar/gpsimd queues, growing to 12-16 segs later.
 - Per block the ps matmuls (gated only on DMA) issue immediately; the
   ps2 matmuls (gated on squares) are deferred one block, and dummy
   filler matmuls pad early DMA gaps: an idle PE drops its p-state
   (57ns -> 108ns per matmul), so the PE is never allowed to idle.
 - Squares split across ACT/DVE/GPSIMD by measured rates (~138/165/270
   ns per 129-elem segment with amortized per-instr overhead).
 - Output [64,130]: ps panel (shipped as soon as ps closes, overlapping
   the ps2 tail) + on-chip row-sums of ps2.
"""

import numpy as np
import ml_dtypes

import concourse.bass as bass
import concourse.mybir as mybir
import concourse.tile as tile
from concourse import bacc
from concourse.bass_utils import run_bass_kernel_spmd

N, K, D = 131072, 64, 128
NCORES = 8
W = D + 1
NS = N // NCORES
SEGS = NS // 128          # 128 segments of 128 rows
BW = W + K                # combined bytes per seg per partition (193)

FP8 = mybir.dt.float8e4
NP_FP8 = ml_dtypes.float8_e4m3

# blocks: (start_seg, n_segs, queue, square slices) in CONSUMPTION order.
# queue: 'sp' (sync), 'act' (scalar), 'gps' (gpsimd).
# Tapered sizes: small head (fast first pair), 16-seg steady state (sem
# granularity tracks the stream), small tail (tiny post-DMA square tail).
# The two sp blocks are consumed near the end but prefetched first on the
# idle sync queue, so ALL gpsimd square work (slow: ~335ns/seg + 500ns
# launch) rides them with ~6us of lead time.
# Listed in CONSUMPTION order (expected readiness order, not seg order).
# FILLERS: dummy matmuls inserted after this block's ps2 group to keep
# the PE busy through the early DMA ramp (an idle PE resets its p-state
# ramp back to 108ns/MM).
# DMA sems fire on the SLOWEST of 16 engine rows; under cross-queue
# contention big early pieces complete several us after most of their
# data lands. So the early window uses small 8-seg pieces round-robined
# across act/gps, growing to 12-16 segs later. sp prefetches the two
# late blocks; all gpsimd square work rides the first one.
BLOCKS = (
    #  seg   n   queue  slices                          fillers
    (0,   2,  "act", (("a", 2),), 10),
    (2,   10, "gps", (("a", 6), ("d", 4)), 12),
    (12,  8,  "act", (("a", 4), ("d", 4)), 6),
    (20,  8,  "gps", (("a", 4), ("d", 4)), 4),
    (28,  8,  "act", (("a", 4), ("d", 4)), 4),
    (36,  8,  "gps", (("a", 4), ("d", 4)), 8),
    (44,  12, "act", (("a", 6), ("d", 6)), 4),
    (56,  12, "gps", (("a", 8), ("d", 4)), 0),
    (68,  12, "act", (("a", 6), ("d", 6)), 0),
    (80,  16, "gps", (("a", 10), ("d", 6)), 0),
    (96,  16, "sp",  (("g", 16),), 0),
    (112, 16, "sp",  (("g", 8), ("a", 4), ("d", 4)), 0),
)
# per-queue DMA issue order (indices into BLOCKS), per-queue FIFO:
# sp: the gpsimd-squared block first (gpsimd is start-gated on it).
ISSUE_ORDER = (10, 0, 1, 2, 3, 11, 4, 5, 6, 7, 8, 9)

N_PE_WARM = 30
N_DVE_WARM = 0   # scheduler floats these mid-stream: they add load, not warmup
N_GPS_WARM = 1   # pulls the Q7 library load off the critical path


def _block_offsets():
    offs = {}
    o = 0
    for bs, bn, q, sl, fl in BLOCKS:
        offs[bs] = o
        o += bn * BW
    assert o == SEGS * BW
    return offs


def build_nc():
    f32 = mybir.dt.float32
    DR = mybir.MatmulPerfMode.DoubleRow

    nc = bacc.Bacc("TRN2", target_bir_lowering=False, debug=False)
    cp = nc.dram_tensor("cp", [128, SEGS * BW], FP8, kind="ExternalInput")
    out = nc.dram_tensor("out", [K, W + 1], f32, kind="ExternalOutput")

    offs = _block_offsets()

    with (
        tile.TileContext(nc) as tc,
        tc.tile_pool(name="cb", bufs=1) as cpool,
        tc.tile_pool(name="scr", bufs=1) as spool,
        tc.tile_pool(name="warm", bufs=3) as wpool,
        tc.tile_pool(name="one", bufs=1) as onepool,
        tc.tile_pool(name="ps", bufs=1, space="PSUM") as pspool,
    ):
        qmap = {"sp": nc.sync, "act": nc.scalar, "gps": nc.gpsimd}
        engmap = {"a": nc.scalar, "d": nc.vector, "g": nc.gpsimd}

        ps = pspool.tile([K, W], f32, tag="ps")
        ps2 = pspool.tile([K, W], f32, tag="ps2")
        wps = pspool.tile([K, W], f32, tag="wps")

        cpt = cpool.tile([128, SEGS * BW], FP8, tag="cp")
        sqt = spool.tile([128, SEGS * W], FP8, tag="sq")

        # warm tiles (memset, no DMA deps)
        wx = wpool.tile([128, 2 * W], FP8, tag="wx")
        wr = wpool.tile([128, 2 * K], FP8, tag="wr")
        wsq = wpool.tile([128, 2 * W], FP8, tag="wsq")
        nc.gpsimd.memset(wx, 0)
        nc.gpsimd.memset(wr, 0)

        # ---- all input DMAs up front ----
        for bi in ISSUE_ORDER:
            bs, bn, q, _, _ = BLOCKS[bi]
            o = offs[bs]
            qmap[q].dma_start(
                out=cpt[:, o:o + bn * BW], in_=cp[:, o:o + bn * BW]
            )

        # ---- engine warmups (pre-data) ----
        wx3 = wx.rearrange("p (s w) -> p s w", w=W)
        wr3 = wr.rearrange("p (s k) -> p s k", k=K)
        for _ in range(N_GPS_WARM):
            nc.gpsimd.tensor_mul(wsq[:, 0:W], wx[:, 0:W], wx[:, 0:W])
        for _ in range(N_DVE_WARM):
            nc.vector.tensor_mul(wsq[:, 0:W], wx[:, 0:W], wx[:, 0:W])
        for _ in range(N_PE_WARM):
            nc.tensor.matmul(
                wps, lhsT=wr3, rhs=wx3, start=True, stop=True, perf_mode=DR
            )

        sq3 = sqt.rearrange("p (s w) -> p s w", w=W)

        # ---- main pipeline, blocks in consumption (arrival) order ----
        # Per block: emit the ps matmuls (gated only on the block's DMA)
        # immediately, and DEFER the ps2 matmuls (gated on squares) by one
        # block. The PE then always has DMA-gated work while squares cook,
        # so it never stalls and its p-state ramp (108ns -> 57ns per MM)
        # is not reset mid-stream.
        def views(bs, bn):
            o = offs[bs]
            xb3 = cpt[:, o:o + bn * W].rearrange("p (s w) -> p s w", w=W)
            rb3 = cpt[:, o + bn * W:o + bn * BW].rearrange(
                "p (s k) -> p s k", k=K
            )
            return xb3, rb3

        nblocks = len(BLOCKS)

        def ps_mms(bi, start):
            bs, bn, _, _, _ = BLOCKS[bi]
            xb3, rb3 = views(bs, bn)
            for j in range(0, bn, 2):
                nc.tensor.matmul(
                    ps,
                    lhsT=rb3[:, j:j + 2, :],
                    rhs=xb3[:, j:j + 2, :],
                    start=start and j == 0,
                    stop=(bi == nblocks - 1) and j == bn - 2,
                    perf_mode=DR,
                )

        def ps2_mms(bi, start):
            bs, bn, _, _, _ = BLOCKS[bi]
            _, rb3 = views(bs, bn)
            for j in range(0, bn, 2):
                nc.tensor.matmul(
                    ps2,
                    lhsT=rb3[:, j:j + 2, :],
                    rhs=sq3[:, bs + j:bs + j + 2, :],
                    start=start and j == 0,
                    stop=(bi == nblocks - 1) and j == bn - 2,
                    perf_mode=DR,
                )

        for bi, (bs, bn, q, slices, fillers) in enumerate(BLOCKS):
            assert sum(n for _, n in slices) == bn
            o = offs[bs]
            a0 = 0
            for ename, n in slices:
                a1 = a0 + n
                eng = engmap[ename]
                src = cpt[:, o + a0 * W:o + a1 * W]
                dst = sqt[:, (bs + a0) * W:(bs + a1) * W]
                if eng is nc.scalar:
                    eng.square(dst, src)
                else:
                    eng.tensor_mul(dst, src, src)
                a0 = a1

            ps_mms(bi, start=(bi == 0))
            if bi > 0:
                ps2_mms(bi - 1, start=(bi == 1))
            for _ in range(fillers):
                nc.tensor.matmul(
                    wps, lhsT=wr3, rhs=wx3, start=True, stop=True,
                    perf_mode=DR,
                )
        ps2_mms(nblocks - 1, start=False)

        # ---- evac + out: ps panel + row-sums of ps2 ----
        osb = onepool.tile([K, W + 1], f32)
        nc.vector.tensor_copy(osb[:, 0:W], ps)
        nc.vector.tensor_reduce(
            osb[:, W:W + 1], ps2, axis=mybir.AxisListType.X,
            op=mybir.AluOpType.add,
        )
        nc.scalar.dma_start(out=out[:, :], in_=osb)

    nc.compile()
    return nc


def make_in_maps(X, r, mus, ncores=NCORES):
    X = np.ascontiguousarray(np.asarray(X, dtype=np.float32))
    r = np.ascontiguousarray(np.asarray(r, dtype=np.float32))
    n = X.shape[0]
    ns = n // ncores

    Xb = X.astype(NP_FP8)
    rb = r.astype(NP_FP8)

    in_maps = []
    for i in range(ncores):
        Xa = np.empty((128, ns // 128, W), NP_FP8)
        Xa[:, :, :D] = Xb[i * ns:(i + 1) * ns].reshape(128, ns // 128, D)
        Xa[:, :, D] = 1.0
        rc = rb[i * ns:(i + 1) * ns].reshape(128, ns // 128, K)

        cpb = np.empty((128, (ns // 128) * BW), NP_FP8)
        o = 0
        for bs, bn, q, sl, fl in BLOCKS:
            cpb[:, o:o + bn * W] = Xa[:, bs:bs + bn].reshape(128, bn * W)
            o += bn * W
            cpb[:, o:o + bn * K] = rc[:, bs:bs + bn].reshape(128, bn * K)
            o += bn * K
        in_maps.append({"cp": np.ascontiguousarray(cpb)})
    return in_maps


def combine_outputs(results, mus):
    mus = np.asarray(mus, dtype=np.float32)
    musq = (mus.astype(np.float64) ** 2).sum(1)
    # out panel: cols 0:W = ps (r.T @ [X|1]); col W = row-sums of ps2,
    # i.e. sum_n r[n,k]*(xsq_n + 1) per k (the +1 from the squared ones
    # column is cancelled by the musq-1 coefficient on ps col 128).
    ma = np.concatenate(
        [-2.0 * mus.astype(np.float64), musq[:, None] - 1.0], axis=1
    )
    total = 0.0
    for res in results:
        panel = res["out"].astype(np.float64)
        total += float((ma * panel[:, :W]).sum()) + float(panel[:, W].sum())
    return np.array(total / (N * K), dtype=np.float32)


def kernel(X, r, mus):
    nc = build_nc()
    in_maps = make_in_maps(X, r, mus)
    res = run_bass_kernel_spmd(nc, in_maps, list(range(NCORES)))
    return combine_outputs(res.results[:NCORES], mus)


# revision 5
# speedup vs baseline: 1.0494x; 1.0494x over previous
"""IsoGMM loss kernel for 8 Trainium2 NeuronCores.

loss = mean_{n,k} r[n,k] * ||X[n] - mus[k]||^2

Decomposition (two accumulated PE matmuls per core over 64 DoubleRow
row-pair segments; fp8 e4m3 inputs, ~4e-3 rel err vs the 2e-2 budget):
  ps  [64,129] += r_pair.T @ [X|1]_pair        (M = r.T X, col 128 = C_k)
  ps2 [64,129] += r_pair.T @ ([X|1]^2)_pair    (row-sum -> T1 + C_k)
Host combine contracts ps with mus (O(K*D)) and reads the on-chip
row-reduction of ps2.

Pipeline structure (from perfetto/ntff trace analysis):
 - Host interleaves X and r into per-block contiguous pieces
   ([X(n x 129) | r(n x 64)] per partition), so one DMA delivers
   everything a run of pairs needs - no X/r queue competition, bigger
   per-partition lines.
 - Blocks are consumed in expected ARRIVAL order, not segment order
   (the loss is a sum; PSUM accumulation order is irrelevant). The sync
   queue prefetches two late blocks early; all gpsimd square work rides
   them (gpsimd needs ~2.8us lead per 12-seg slice).
 - DMA sems fire on the slowest of 16 engine rows, so under cross-queue
   contention the early window uses small 8-seg pieces round-robined
   across the scalar/gpsimd queues, growing to 12-16 segs later.
 - Per block the ps matmuls (gated only on DMA) issue immediately; the
   ps2 matmuls (gated on squares) are deferred one block, and dummy
   filler matmuls pad early DMA gaps: an idle PE drops its p-state
   (57ns -> 108ns per matmul), so the PE is never allowed to idle.
 - Squares split across ACT/DVE/GPSIMD by measured rates (~138/165/270
   ns per 129-elem segment with amortized per-instr overhead).
 - Output [64,130]: ps panel (shipped as soon as ps closes, overlapping
   the ps2 tail) + on-chip row-sums of ps2.
"""

import numpy as np
import ml_dtypes

import concourse.bass as bass
import concourse.mybir as mybir
import concourse.tile as tile
from concourse import bacc
from concourse.bass_utils import run_bass_kernel_spmd

N, K, D = 131072, 64, 128
NCORES = 8
W = D + 1
NS = N // NCORES
SEGS = NS // 128          # 128 segments of 128 rows
BW = W + K                # combined bytes per seg per partition (193)

FP8 = mybir.dt.float8e4
NP_FP8 = ml_dtypes.float8_e4m3

# blocks: (start_seg, n_segs, queue, square slices) in CONSUMPTION order.
# queue: 'sp' (sync), 'act' (scalar), 'gps' (gpsimd).
# Tapered sizes: small head (fast first pair), 16-seg steady state (sem
# granularity tracks the stream), small tail (tiny post-DMA square tail).
# The two sp blocks are consumed near the end but prefetched first on the
# idle sync queue, so ALL gpsimd square work (slow: ~335ns/seg + 500ns
# launch) rides them with ~6us of lead time.
# Listed in CONSUMPTION order (expected readiness order, not seg order).
# FILLERS: dummy matmuls inserted after this block's ps2 group to keep
# the PE busy through the early DMA ramp (an idle PE resets its p-state
# ramp back to 108ns/MM).
# DMA sems fire on the SLOWEST of 16 engine rows; under cross-queue
# contention big early pieces complete several us after most of their
# data lands. So the early window uses small 8-seg pieces round-robined
# across act/gps, growing to 12-16 segs later. sp prefetches the two
# late blocks; all gpsimd square work rides the first one.
BLOCKS = (
    #  seg   n   queue  slices                          fillers
    (0,   2,  "act", (("a", 1), ("d", 1)), 10),
    (2,   10, "gps", (("a", 6), ("d", 4)), 12),
    (12,  8,  "act", (("a", 4), ("d", 4)), 6),
    (20,  8,  "gps", (("a", 4), ("d", 4)), 4),
    (28,  8,  "act", (("a", 4), ("d", 4)), 4),
    (36,  8,  "gps", (("a", 4), ("d", 4)), 8),
    (44,  12, "act", (("a", 6), ("d", 6)), 4),
    (56,  12, "gps", (("a", 8), ("d", 4)), 0),
    (68,  12, "act", (("a", 6), ("d", 6)), 0),
    (80,  16, "gps", (("a", 10), ("d", 6)), 0),
    (96,  16, "sp",  (("g", 16),), 0),
    (112, 16, "sp",  (("g", 8), ("a", 4), ("d", 4)), 0),
)
# per-queue DMA issue order (indices into BLOCKS), per-queue FIFO:
# sp: the gpsimd-squared block first (gpsimd is start-gated on it).
ISSUE_ORDER = (10, 0, 1, 2, 3, 11, 4, 5, 6, 7, 8, 9)

N_PE_WARM = 30
N_DVE_WARM = 0   # scheduler floats these mid-stream: they add load, not warmup
N_GPS_WARM = 1   # pulls the Q7 library load off the critical path


def _block_offsets():
    offs = {}
    o = 0
    for bs, bn, q, sl, fl in BLOCKS:
        offs[bs] = o
        o += bn * BW
    assert o == SEGS * BW
    return offs


def build_nc():
    f32 = mybir.dt.float32
    DR = mybir.MatmulPerfMode.DoubleRow

    nc = bacc.Bacc("TRN2", target_bir_lowering=False, debug=False)
    cp = nc.dram_tensor("cp", [128, SEGS * BW], FP8, kind="ExternalInput")
    out = nc.dram_tensor("out", [K, W + 1], f32, kind="ExternalOutput")

    offs = _block_offsets()

    with (
        tile.TileContext(nc) as tc,
        tc.tile_pool(name="cb", bufs=1) as cpool,
        tc.tile_pool(name="scr", bufs=1) as spool,
        tc.tile_pool(name="warm", bufs=3) as wpool,
        tc.tile_pool(name="one", bufs=1) as onepool,
        tc.tile_pool(name="ps", bufs=1, space="PSUM") as pspool,
    ):
        qmap = {"sp": nc.sync, "act": nc.scalar, "gps": nc.gpsimd}
        engmap = {"a": nc.scalar, "d": nc.vector, "g": nc.gpsimd}

        ps = pspool.tile([K, W], f32, tag="ps")
        ps2 = pspool.tile([K, W], f32, tag="ps2")
        wps = pspool.tile([K, W], f32, tag="wps")

        cpt = cpool.tile([128, SEGS * BW], FP8, tag="cp")
        sqt = spool.tile([128, SEGS * W], FP8, tag="sq")

        # warm tiles (memset, no DMA deps)
        wx = wpool.tile([128, 2 * W], FP8, tag="wx")
        wr = wpool.tile([128, 2 * K], FP8, tag="wr")
        wsq = wpool.tile([128, 2 * W], FP8, tag="wsq")
        nc.gpsimd.memset(wx, 0)
        nc.gpsimd.memset(wr, 0)

        # ---- all input DMAs up front ----
        for bi in ISSUE_ORDER:
            bs, bn, q, _, _ = BLOCKS[bi]
            o = offs[bs]
            qmap[q].dma_start(
                out=cpt[:, o:o + bn * BW], in_=cp[:, o:o + bn * BW]
            )

        # ---- engine warmups (pre-data) ----
        wx3 = wx.rearrange("p (s w) -> p s w", w=W)
        wr3 = wr.rearrange("p (s k) -> p s k", k=K)
        for _ in range(N_GPS_WARM):
            nc.gpsimd.tensor_mul(wsq[:, 0:W], wx[:, 0:W], wx[:, 0:W])
        for _ in range(N_DVE_WARM):
            nc.vector.tensor_mul(wsq[:, 0:W], wx[:, 0:W], wx[:, 0:W])
        for _ in range(N_PE_WARM):
            nc.tensor.matmul(
                wps, lhsT=wr3, rhs=wx3, start=True, stop=True, perf_mode=DR
            )

        sq3 = sqt.rearrange("p (s w) -> p s w", w=W)

        # ---- main pipeline, blocks in consumption (arrival) order ----
        # Per block: emit the ps matmuls (gated only on the block's DMA)
        # immediately, and DEFER the ps2 matmuls (gated on squares) by one
        # block. The PE then always has DMA-gated work while squares cook,
        # so it never stalls and its p-state ramp (108ns -> 57ns per MM)
        # is not reset mid-stream.
        def views(bs, bn):
            o = offs[bs]
            xb3 = cpt[:, o:o + bn * W].rearrange("p (s w) -> p s w", w=W)
            rb3 = cpt[:, o + bn * W:o + bn * BW].rearrange(
                "p (s k) -> p s k", k=K
            )
            return xb3, rb3

        nblocks = len(BLOCKS)

        def ps_mms(bi, start):
            bs, bn, _, _, _ = BLOCKS[bi]
            xb3, rb3 = views(bs, bn)
            for j in range(0, bn, 2):
                nc.tensor.matmul(
                    ps,
                    lhsT=rb3[:, j:j + 2, :],
                    rhs=xb3[:, j:j + 2, :],
                    start=start and j == 0,
                    stop=(bi == nblocks - 1) and j == bn - 2,
                    perf_mode=DR,
                )

        def ps2_mms(bi, start):
            bs, bn, _, _, _ = BLOCKS[bi]
            _, rb3 = views(bs, bn)
            for j in range(0, bn, 2):
                nc.tensor.matmul(
                    ps2,
                    lhsT=rb3[:, j:j + 2, :],
                    rhs=sq3[:, bs + j:bs + j + 2, :],
                    start=start and j == 0,
                    stop=(bi == nblocks - 1) and j == bn - 2,
                    perf_mode=DR,
                )

        for bi, (bs, bn, q, slices, fillers) in enumerate(BLOCKS):
            assert sum(n for _, n in slices) == bn
            o = offs[bs]
            a0 = 0
            for ename, n in slices:
                a1 = a0 + n
                eng = engmap[ename]
                src = cpt[:, o + a0 * W:o + a1 * W]
                dst = sqt[:, (bs + a0) * W:(bs + a1) * W]
                if eng is nc.scalar:
                    eng.square(dst, src)
                else:
                    eng.tensor_mul(dst, src, src)
                a0 = a1

            ps_mms(bi, start=(bi == 0))
            if bi > 0:
                ps2_mms(bi - 1, start=(bi == 1))
            for _ in range(fillers):
                nc.tensor.matmul(
                    wps, lhsT=wr3, rhs=wx3, start=True, stop=True,
                    perf_mode=DR,
                )
        ps2_mms(nblocks - 1, start=False)

        # ---- evac + out: ps panel + row-sums of ps2 ----
        osb = onepool.tile([K, W + 1], f32)
        nc.vector.tensor_copy(osb[:, 0:W], ps)
        nc.vector.tensor_reduce(
            osb[:, W:W + 1], ps2, axis=mybir.AxisListType.X,
            op=mybir.AluOpType.add,
        )
        nc.scalar.dma_start(out=out[:, :], in_=osb)

    nc.compile()
    return nc


def make_in_maps(X, r, mus, ncores=NCORES):
    X = np.ascontiguousarray(np.asarray(X, dtype=np.float32))
    r = np.ascontiguousarray(np.asarray(r, dtype=np.float32))
    n = X.shape[0]
    ns = n // ncores

    Xb = X.astype(NP_FP8)
    rb = r.astype(NP_FP8)

    in_maps = []
    for i in range(ncores):
        Xa = np.empty((128, ns // 128, W), NP_FP8)
        Xa[:, :, :D] = Xb[i * ns:(i + 1) * ns].reshape(128, ns // 128, D)
        Xa[:, :, D] = 1.0
        rc = rb[i * ns:(i + 1) * ns].reshape(128, ns // 128, K)

        cpb = np.empty((128, (ns // 128) * BW), NP_FP8)
        o = 0
        for bs, bn, q, sl, fl in BLOCKS:
            cpb[:, o:o + bn * W] = Xa[:, bs:bs + bn].reshape(128, bn * W)
            o += bn * W
            cpb[:, o:o + bn * K] = rc[:, bs:bs + bn].reshape(128, bn * K)
            o += bn * K
        in_maps.append({"cp": np.ascontiguousarray(cpb)})
    return in_maps


def combine_outputs(results, mus):
    mus = np.asarray(mus, dtype=np.float32)
    musq = (mus.astype(np.float64) ** 2).sum(1)
    # out panel: cols 0:W = ps (r.T @ [X|1]); col W = row-sums of ps2,
    # i.e. sum_n r[n,k]*(xsq_n + 1) per k (the +1 from the squared ones
    # column is cancelled by the musq-1 coefficient on ps col 128).
    ma = np.concatenate(
        [-2.0 * mus.astype(np.float64), musq[:, None] - 1.0], axis=1
    )
    total = 0.0
    for res in results:
        panel = res["out"].astype(np.float64)
        total += float((ma * panel[:, :W]).sum()) + float(panel[:, W].sum())
    return np.array(total / (N * K), dtype=np.float32)


def kernel(X, r, mus):
    nc = build_nc()
    in_maps = make_in_maps(X, r, mus)
    res = run_bass_kernel_spmd(nc, in_maps, list(range(NCORES)))
    return combine_outputs(res.results[:NCORES], mus)
